# revision 24
# baseline (speedup 1.0000x reference)
"""Fused GPT-2 transformer block on 8 Trainium2 NeuronCores.

Sharding: 8 cores = 4 batches x 2 causal-balanced folds. Core (b, f) owns the 8
interleaved 128-token blocks of parity f of batch b (queries), and receives all
2048 tokens of batch b as context, permuted [other-parity blocks | own blocks].
Causality is enforced by a per-core additive mask shipped as data, so a single
SPMD program serves all cores. No collectives.

Layouts: LN1(x) is PE-transposed to hT [D, tok]; Q/K are produced directly in
head-major transposed layout, V in token-major layout with an appended ones
column (so the P@V matmul also accumulates softmax denominators). Attention
runs fully in the transposed layout; proj/fc2 contract against feature-major
lhsT slices, landing outputs back in token-major layout for residuals/LN.
All matmuls run in float32r (full PE rate, ~1.5e-4 rel err).
"""

import contextlib
import os

import numpy as np

import concourse.bass as bass
import concourse.mybir as mybir
import concourse.tile as tile
from concourse import bacc
from concourse.bass_utils import run_bass_kernel_spmd
from concourse.masks import make_identity

F32 = mybir.dt.float32
F32R = mybir.dt.float32r
AF = mybir.ActivationFunctionType
ALU = mybir.AluOpType

B, S, D, H = 4, 2048, 1024, 16
HD = D // H          # 64
DFF = 4 * D          # 4096
EPS = 1e-5
MASKED_BIAS = -10000.0
N_CORES = 8

SB = S // 128        # 16 ctx blocks
OWN = S // 2         # 1024 own tokens
OB = OWN // 128      # 8 own blocks
NQG = 4              # q-groups of 256
QG = 256
HSETS = 4            # head sets
HPS = H // HSETS     # 4 heads per set


def _klist(g):
    """ctx k-block indices computed for q-group g (own blocks 2g, 2g+1)."""
    return list(range(0, 2 * g + 2)) + list(range(8, 8 + 2 * g + 2))


def build_nc(am_zero=True):
    nc = bacc.Bacc("TRN2", target_bir_lowering=False, debug=False,
                   num_devices=N_CORES)

    X = nc.dram_tensor("X", [S, D], F32, kind="ExternalInput")
    MSK = (nc.dram_tensor("MSK", [2, 128, 512], F32, kind="ExternalInput")
           if am_zero else
           nc.dram_tensor("MSK", [16, 128, QG], F32, kind="ExternalInput"))
    AM = nc.dram_tensor("AM", [128, SB], F32, kind="ExternalInput")
    WQ = nc.dram_tensor("WQ", [D, D], F32, kind="ExternalInput")
    WK = nc.dram_tensor("WK", [D, D], F32, kind="ExternalInput")
    WV = nc.dram_tensor("WV", [D, D], F32, kind="ExternalInput")
    BQ = nc.dram_tensor("BQ", [D, 1], F32, kind="ExternalInput")
    BK = nc.dram_tensor("BK", [D, 1], F32, kind="ExternalInput")
    BV = nc.dram_tensor("BV", [1, D], F32, kind="ExternalInput")
    WP = nc.dram_tensor("WP", [D, D], F32, kind="ExternalInput")
    BP = nc.dram_tensor("BP", [1, D], F32, kind="ExternalInput")
    WF = nc.dram_tensor("WF", [D, DFF], F32, kind="ExternalInput")
    BF = nc.dram_tensor("BF", [DFF, 1], F32, kind="ExternalInput")
    WF2 = nc.dram_tensor("WF2", [DFF, D], F32, kind="ExternalInput")
    BF2 = nc.dram_tensor("BF2", [1, D], F32, kind="ExternalInput")
    OUT = nc.dram_tensor("OUT", [OWN, D], F32, kind="ExternalOutput")

    with tile.TileContext(nc) as tc:
        _body(nc, tc, X, MSK, AM, WQ, WK, WV, BQ, BK, BV, WP, BP, WF, BF,
              WF2, BF2, OUT, am_zero)
    nc.compile()
    return nc


def _layernorm_tile(nc, stat, src_tile):
    """In-place LN (no affine) of src_tile [128, D]."""
    sub = 512
    nsub = D // sub
    xs = src_tile[:, :].rearrange("p (n s) -> p n s", s=sub)
    stats = stat.tile([128, nsub, nc.vector.BN_STATS_DIM], F32, tag="bnst")
    for j in range(nsub):
        nc.vector.bn_stats(out=stats[:, j, :], in_=xs[:, j, :])
    mv = stat.tile([128, nc.vector.BN_AGGR_DIM], F32, tag="bnag")
    nc.vector.bn_aggr(out=mv[:, :], in_=stats[:, :, :])
    eps_t = stat.tile([128, 1], F32, tag="eps")
    nc.vector.memset(eps_t[:], EPS)
    nc.scalar.activation(out=mv[:, 1:2], in_=mv[:, 1:2], func=AF.Sqrt,
                         bias=eps_t[:], scale=1.0)
    nc.vector.reciprocal(out=mv[:, 1:2], in_=mv[:, 1:2])
    nc.vector.tensor_scalar(out=src_tile[:, :], in0=src_tile[:, :],
                            scalar1=mv[:, 0:1], scalar2=mv[:, 1:2],
                            op0=ALU.subtract, op1=ALU.mult)


def _body(nc, tc, X, MSK, AM, WQ, WK, WV, BQ, BK, BV, WP, BP, WF, BF, WF2,
          BF2, OUT, am_zero=True):
    PL = int(os.environ.get("KPHASES", "9"))
    CH = 8  # exp-staging chunk, in k-tiles
    with contextlib.ExitStack() as top:
        cst = top.enter_context(tc.tile_pool(name="cst", bufs=1))
        stat = top.enter_context(tc.tile_pool(name="stat", bufs=2))

        ident = cst.tile([128, 128], F32)
        make_identity(nc, ident[:])
        ones_f = cst.tile([1, 128], F32)
        nc.vector.memset(ones_f[:], 1.0)
        ones_c3 = cst.tile([128, HPS, 1], F32)
        nc.vector.memset(ones_c3[:], 1.0)
        ones_r = cst.tile([1, 128], F32R)   # bias-row lhsT
        nc.scalar.copy(ones_r[:], ones_f[:])
        ones_b = cst.tile([1, 64], F32R)    # denominator-broadcast lhsT
        nc.scalar.copy(ones_b[:], ones_f[:, 0:64])

        with contextlib.ExitStack() as attn_stack:
            atp = attn_stack.enter_context(tc.tile_pool(name="atp", bufs=1))
            aT = [atp.tile([128, OWN], F32R, tag=f"aT{p}", name=f"aT{p}")
                  for p in range(8)]

            with contextlib.ExitStack() as ht_stack:
                ht = ht_stack.enter_context(tc.tile_pool(name="ht", bufs=1))

                # ---- Phase 1: LN1 over ctx + transpose -> hT ----
                hT = [ht.tile([128, S], F32R, tag=f"hT{db}", name=f"hT{db}")
                      for db in range(8)]
                with tc.tile_pool(name="psT", bufs=4, space="PSUM") as psT, \
                     tc.tile_pool(name="xin1", bufs=3) as xin:
                    for tb in range(SB):
                        x_t = xin.tile([128, D], F32, tag="x1")
                        nc.sync.dma_start(x_t[:], X[tb * 128:(tb + 1) * 128, :])
                        _layernorm_tile(nc, stat, x_t)
                        for db in range(8):
                            pt = psT.tile([128, 128], F32, tag="tp")
                            nc.tensor.transpose(
                                pt[:], x_t[:, db * 128:(db + 1) * 128],
                                ident[:])
                            nc.vector.tensor_copy(
                                hT[db][:, tb * 128:(tb + 1) * 128], pt[:])

                if PL < 2:
                    return
                with contextlib.ExitStack() as hs_stack:
                    kvq = hs_stack.enter_context(
                        tc.tile_pool(name="kvq", bufs=1))
                    mskp = hs_stack.enter_context(
                        tc.tile_pool(name="mskp", bufs=1))
                    att = hs_stack.enter_context(
                        tc.tile_pool(name="att", bufs=2))
                    wst = hs_stack.enter_context(
                        tc.tile_pool(name="wstA", bufs=1))
                    psKV = hs_stack.enter_context(
                        tc.tile_pool(name="psKV", bufs=2, space="PSUM"))
                    psS = hs_stack.enter_context(
                        tc.tile_pool(name="psS", bufs=2, space="PSUM"))
                    psA = hs_stack.enter_context(
                        tc.tile_pool(name="psA", bufs=2, space="PSUM"))

                    # masks + attention-mask bias, loaded once
                    if am_zero:
                        mskB = mskp.tile([128, 512], F32, tag="mskB",
                                         name="mskB")
                        nc.sync.dma_start(mskB[:], MSK[0, :, :])
                        mskD = mskp.tile([128, 512], F32, tag="mskD",
                                         name="mskD")
                        nc.sync.dma_start(mskD[:], MSK[1, :, :])
                        msk_t = None
                    else:
                        msk_t = []
                        for m in range(16):
                            m_t = mskp.tile([128, QG], F32, tag=f"msk{m}",
                                            name=f"msk{m}")
                            nc.sync.dma_start(m_t[:], MSK[m, :, :])
                            msk_t.append(m_t)
                    am_sb = mskp.tile([128, SB], F32, tag="am", name="am")
                    nc.sync.dma_start(am_sb[:], AM[:, :])

                    for hs in range(HSETS):
                        # ---- Phase 2: K/V/Q projections for this head set ----
                        kT = [kvq.tile([128, S], F32R, tag=f"kT{p}",
                                       name=f"kT{p}") for p in range(2)]
                        qT = [kvq.tile([128, OWN], F32R, tag=f"qT{p}",
                                       name=f"qT{p}") for p in range(2)]
                        vS = [kvq.tile([128, HPS, HD + 1], F32R,
                                       tag=f"vS{tb}", name=f"vS{tb}")
                              for tb in range(SB)]

                        for p in range(2):
                            fcol = hs * 256 + p * 128
                            bq_c = stat.tile([128, 1], F32, tag="bqc")
                            nc.sync.dma_start(bq_c[:], BQ[fcol:fcol + 128, :])
                            bk_c = stat.tile([128, 1], F32, tag="bkc")
                            nc.sync.dma_start(bk_c[:], BK[fcol:fcol + 128, :])
                            wkt, wqt = [], []
                            for db in range(8):
                                w_t = wst.tile([128, 128], F32R,
                                               tag=f"wk{db}", name=f"wk{db}")
                                nc.sync.dma_start(
                                    w_t[:],
                                    WK[db * 128:(db + 1) * 128,
                                       fcol:fcol + 128].bitcast(F32R))
                                wkt.append(w_t)
                                w_t = wst.tile([128, 128], F32R,
                                               tag=f"wq{db}", name=f"wq{db}")
                                nc.sync.dma_start(
                                    w_t[:],
                                    WQ[db * 128:(db + 1) * 128,
                                       fcol:fcol + 128].bitcast(F32R))
                                wqt.append(w_t)
                            for tg in range(4):
                                ps = psKV.tile([128, 512], F32, tag="pk")
                                for db in range(8):
                                    nc.tensor.matmul(
                                        ps[:], wkt[db][:],
                                        hT[db][:, tg * 512:(tg + 1) * 512],
                                        start=(db == 0), stop=(db == 7))
                                nc.vector.tensor_scalar_add(
                                    out=kT[p][:, tg * 512:(tg + 1) * 512],
                                    in0=ps[:], scalar1=bk_c[:])
                            for tg in range(2):
                                ps = psKV.tile([128, 512], F32, tag="pk")
                                for db in range(8):
                                    nc.tensor.matmul(
                                        ps[:], wqt[db][:],
                                        hT[db][:, OWN + tg * 512:
                                               OWN + (tg + 1) * 512],
                                        start=(db == 0), stop=(db == 7))
                                nc.vector.tensor_scalar_add(
                                    out=qT[p][:, tg * 512:(tg + 1) * 512],
                                    in0=ps[:], scalar1=bq_c[:])

                        wvt = []
                        for db in range(8):
                            w_t = wst.tile([128, HPS * HD], F32R,
                                           tag=f"wv{db}", name=f"wv{db}")
                            nc.sync.dma_start(
                                w_t[:],
                                WV[db * 128:(db + 1) * 128,
                                   hs * 256:(hs + 1) * 256].bitcast(F32R))
                            wvt.append(w_t)
                        bv_t = wst.tile([1, HPS * HD], F32R, tag="bv")
                        nc.sync.dma_start(
                            bv_t[:],
                            BV[0:1, hs * 256:(hs + 1) * 256].bitcast(F32R))
                        for tb in range(SB):
                            ps = psKV.tile([128, HPS * HD], F32, tag="pv")
                            for db in range(8):
                                nc.tensor.matmul(
                                    ps[:], hT[db][:, tb * 128:(tb + 1) * 128],
                                    wvt[db][:], start=(db == 0), stop=False)
                            nc.tensor.matmul(ps[:], ones_r[:], bv_t[:],
                                             start=False, stop=True)
                            nc.vector.tensor_copy(
                                vS[tb][:, :, 0:HD],
                                ps[:].rearrange("p (h d) -> p h d", d=HD))
                            nc.scalar.copy(vS[tb][:, :, HD:HD + 1],
                                           ones_c3[:])

                        # ---- Phase 3: attention for this head set ----
                        for g in range(NQG):
                            kl = _klist(g)
                            # masked k-tiles for this q-group -> MSK index
                            mrel = {2 * g: 0, 2 * g + 1: 1,
                                    8 + 2 * g: 2, 9 + 2 * g: 3}
                            for h in range(HPS):
                                p, sub = h // 2, h % 2
                                pa = psA.tile([HD + 1, QG], F32, tag="pa")
                                nchunks = (len(kl) + CH - 1) // CH
                                for c in range(nchunks):
                                    chunk = kl[c * CH:(c + 1) * CH]
                                    wide = att.tile([128, CH * QG], F32R,
                                                    tag="wide", name="wide")
                                    if am_zero:
                                        for pi in range(len(chunk) // 2):
                                            kb0 = chunk[2 * pi]
                                            pss = psS.tile([128, 2 * QG], F32,
                                                           tag="ps")
                                            for u in range(2):
                                                kb = chunk[2 * pi + u]
                                                nc.tensor.matmul(
                                                    pss[:, u * QG:
                                                        (u + 1) * QG],
                                                    kT[p][sub * 64:
                                                          (sub + 1) * 64,
                                                          kb * 128:
                                                          (kb + 1) * 128],
                                                    qT[p][sub * 64:
                                                          (sub + 1) * 64,
                                                          g * QG:(g + 1) * QG],
                                                    start=True, stop=True)
                                            ws = wide[:, 2 * pi * QG:
                                                      (2 * pi + 2) * QG]
                                            if kb0 == 2 * g:
                                                nc.vector.scalar_tensor_tensor(
                                                    out=ws, in0=pss[:],
                                                    scalar=0.0, in1=mskB[:],
                                                    op0=ALU.add, op1=ALU.add)
                                            elif kb0 == 8 + 2 * g:
                                                nc.vector.scalar_tensor_tensor(
                                                    out=ws, in0=pss[:],
                                                    scalar=0.0, in1=mskD[:],
                                                    op0=ALU.add, op1=ALU.add)
                                            else:
                                                nc.vector.tensor_copy(
                                                    ws, pss[:])
                                    else:
                                        for i, kb in enumerate(chunk):
                                            pss = psS.tile([128, QG], F32,
                                                           tag="ps")
                                            nc.tensor.matmul(
                                                pss[:],
                                                kT[p][sub * 64:(sub + 1) * 64,
                                                      kb * 128:
                                                      (kb + 1) * 128],
                                                qT[p][sub * 64:(sub + 1) * 64,
                                                      g * QG:(g + 1) * QG],
                                                start=True, stop=True)
                                            wslice = wide[:, i * QG:
                                                          (i + 1) * QG]
                                            if kb in mrel:
                                                nc.vector.scalar_tensor_tensor(
                                                    out=wslice, in0=pss[:],
                                                    scalar=am_sb[:, kb:kb + 1],
                                                    in1=msk_t[g * 4
                                                              + mrel[kb]][:],
                                                    op0=ALU.add, op1=ALU.add)
                                            else:
                                                nc.vector.tensor_scalar_add(
                                                    out=wslice, in0=pss[:],
                                                    scalar1=am_sb[:,
                                                                  kb:kb + 1])
                                    nw = len(chunk) * QG
                                    nc.scalar.activation(
                                        wide[:, 0:nw],
                                        wide[:, 0:nw].bitcast(F32), AF.Exp)
                                    for i, kb in enumerate(chunk):
                                        nc.tensor.matmul(
                                            pa[:], vS[kb][:, h, :],
                                            wide[:, i * QG:(i + 1) * QG],
                                            start=(c == 0 and i == 0),
                                            stop=(c == nchunks - 1
                                                  and i == len(chunk) - 1))
                                rec = att.tile([1, QG], F32R, tag="rec")
                                with nc.allow_low_precision(
                                        reason="softmax denom reciprocal, "
                                               "f32r is ~fp32"):
                                    nc.vector.reciprocal(rec[:],
                                                         pa[HD:HD + 1, :])
                                pb = psS.tile([64, QG], F32, tag="ps")
                                nc.tensor.matmul(pb[:], ones_b[:], rec[:],
                                                 start=True, stop=True)
                                bc = att.tile([64, QG], F32, tag="bc")
                                nc.scalar.copy(bc[:], pb[:])
                                ap_idx = 2 * hs + p
                                nc.vector.tensor_mul(
                                    aT[ap_idx][sub * 64:(sub + 1) * 64,
                                               g * QG:(g + 1) * QG],
                                    pa[0:HD, :], bc[:])

            if PL < 4:
                return
            # ---- Phase 4: proj + residual -> x2 ----
            x2p = top.enter_context(tc.tile_pool(name="x2p", bufs=1,
                                                 side="right"))
            x2 = [x2p.tile([128, D], F32, tag=f"x2{tb}", name=f"x2{tb}")
                  for tb in range(OB)]
            with tc.tile_pool(name="psP", bufs=2, space="PSUM") as psP, \
                 tc.tile_pool(name="wstP", bufs=1) as wst, \
                 tc.tile_pool(name="xqp", bufs=1) as xqp:
                xqs = []
                for tb in range(OB):
                    xq_t = xqp.tile([128, D], F32, tag=f"xq{tb}",
                                    name=f"xq{tb}")
                    nc.sync.dma_start(
                        xq_t[:], X[OWN + tb * 128:OWN + (tb + 1) * 128, :])
                    xqs.append(xq_t)
                for fg in range(2):
                    wpt = []
                    for ab in range(8):
                        w_t = wst.tile([128, 512], F32R, tag=f"wp{ab}",
                                       name=f"wp{ab}")
                        nc.sync.dma_start(
                            w_t[:],
                            WP[ab * 128:(ab + 1) * 128,
                               fg * 512:(fg + 1) * 512].bitcast(F32R))
                        wpt.append(w_t)
                    bp_t = wst.tile([1, 512], F32R, tag="bp")
                    nc.sync.dma_start(
                        bp_t[:],
                        BP[0:1, fg * 512:(fg + 1) * 512].bitcast(F32R))
                    for tb in range(OB):
                        ps = psP.tile([128, 512], F32, tag="pp")
                        for ab in range(8):
                            nc.tensor.matmul(
                                ps[:], aT[ab][:, tb * 128:(tb + 1) * 128],
                                wpt[ab][:], start=(ab == 0), stop=False)
                        nc.tensor.matmul(ps[:], ones_r[:], bp_t[:],
                                         start=False, stop=True)
                        nc.vector.tensor_add(
                            x2[tb][:, fg * 512:(fg + 1) * 512], ps[:],
                            xqs[tb][:, fg * 512:(fg + 1) * 512])

        if PL < 5:
            return
        # ---- Phase 5: LN2 + transpose -> h2T ----
        with contextlib.ExitStack() as mlp_stack:
            ht2 = mlp_stack.enter_context(tc.tile_pool(name="ht2", bufs=1))
            h2T = [ht2.tile([128, OWN], F32R, tag=f"h2T{db}", name=f"h2T{db}")
                   for db in range(8)]
            with tc.tile_pool(name="psT2", bufs=4, space="PSUM") as psT2, \
                 tc.tile_pool(name="xin5", bufs=3) as xin:
                for tb in range(OB):
                    x_t = xin.tile([128, D], F32, tag="x1")
                    nc.vector.tensor_copy(x_t[:], x2[tb][:])
                    _layernorm_tile(nc, stat, x_t)
                    for db in range(8):
                        pt = psT2.tile([128, 128], F32, tag="tp")
                        nc.tensor.transpose(
                            pt[:], x_t[:, db * 128:(db + 1) * 128], ident[:])
                        nc.vector.tensor_copy(
                            h2T[db][:, tb * 128:(tb + 1) * 128], pt[:])

            if PL < 6:
                return
            # ---- Phase 6: MLP ----
            with contextlib.ExitStack() as mlp2:
                gtp = mlp2.enter_context(tc.tile_pool(name="gtp", bufs=1))
                wst = mlp2.enter_context(tc.tile_pool(name="wstF", bufs=2))
                wst6 = mlp2.enter_context(tc.tile_pool(name="wstF6", bufs=6))
                outp = mlp2.enter_context(tc.tile_pool(name="outp", bufs=3))
                psF = mlp2.enter_context(
                    tc.tile_pool(name="psF", bufs=2, space="PSUM"))
                psO = mlp2.enter_context(
                    tc.tile_pool(name="psO", bufs=1, space="PSUM"))
                for tg in range(2):
                    gt = [gtp.tile([128, 512], F32R, tag=f"gt{j}",
                                   name=f"gt{j}") for j in range(32)]
                    for jj in range(8):      # groups of 4 dff blocks
                        slabs = []
                        for db in range(8):
                            w_t = wst.tile([128, 512], F32R, tag=f"wf{db}",
                                           name=f"wf{db}")
                            nc.sync.dma_start(
                                w_t[:],
                                WF[db * 128:(db + 1) * 128,
                                   jj * 512:(jj + 1) * 512].bitcast(F32R))
                            slabs.append(w_t)
                        for sj in range(4):
                            j = jj * 4 + sj
                            bf_c = stat.tile([128, 1], F32, tag="bfc")
                            nc.sync.dma_start(bf_c[:],
                                              BF[j * 128:(j + 1) * 128, :])
                            ps = psF.tile([128, 512], F32, tag="pf")
                            for db in range(8):
                                nc.tensor.matmul(
                                    ps[:],
                                    slabs[db][:, sj * 128:(sj + 1) * 128],
                                    h2T[db][:, tg * 512:(tg + 1) * 512],
                                    start=(db == 0), stop=(db == 7))
                            nc.scalar.activation(gt[j][:], ps[:],
                                                 AF.Gelu_apprx_tanh,
                                                 bias=bf_c[:], scale=1.0)
                    for fg in range(2):
                        pso = [psO.tile([128, 512], F32, tag=f"po{tb}",
                                        name=f"po{tb}") for tb in range(4)]
                        for j in range(32):
                            w_t = wst6.tile([128, 512], F32R, tag="wf2",
                                            name="wf2")
                            nc.sync.dma_start(
                                w_t[:],
                                WF2[j * 128:(j + 1) * 128,
                                    fg * 512:(fg + 1) * 512].bitcast(F32R))
                            for tb in range(4):
                                nc.tensor.matmul(
                                    pso[tb][:],
                                    gt[j][:, tb * 128:(tb + 1) * 128],
                                    w_t[:], start=(j == 0), stop=False)
                        bf2_t = wst.tile([1, 512], F32R, tag="bf2")
                        nc.sync.dma_start(
                            bf2_t[:],
                            BF2[0:1, fg * 512:(fg + 1) * 512].bitcast(F32R))
                        for tb in range(4):
                            nc.tensor.matmul(pso[tb][:], ones_r[:], bf2_t[:],
                                             start=False, stop=True)
                            o_t = outp.tile([128, 512], F32, tag="ot")
                            gtb = tg * 4 + tb
                            nc.vector.tensor_add(
                                o_t[:], pso[tb][:],
                                x2[gtb][:, fg * 512:(fg + 1) * 512])
                            nc.sync.dma_start(
                                OUT[gtb * 128:(gtb + 1) * 128,
                                    fg * 512:(fg + 1) * 512], o_t[:])


_NC_CACHE = {}


def _get_nc(am_zero=True):
    key = f"nc{int(am_zero)}"
    if key not in _NC_CACHE:
        _NC_CACHE[key] = build_nc(am_zero)
    return _NC_CACHE[key]


def _perm_for(f):
    other = [2 * j + (1 - f) for j in range(8)]
    own = [2 * j + f for j in range(8)]
    blocks = other + own
    return np.concatenate([np.arange(b * 128, (b + 1) * 128) for b in blocks])


def make_in_maps(hidden_states, attention_mask, ln1_g, ln1_b, W_attn, b_attn,
                 W_proj, b_proj, ln2_g, ln2_b, W_fc, b_fc, W_fc2, b_fc2):
    f32 = lambda a: np.asarray(a, dtype=np.float32)
    hidden_states = f32(hidden_states)
    attention_mask = f32(attention_mask)
    ln1_g, ln1_b = f32(ln1_g), f32(ln1_b)
    ln2_g, ln2_b = f32(ln2_g), f32(ln2_b)
    W_attn, b_attn = f32(W_attn), f32(b_attn)
    W_proj, b_proj = f32(W_proj), f32(b_proj)
    W_fc, b_fc = f32(W_fc), f32(b_fc)
    W_fc2, b_fc2 = f32(W_fc2), f32(b_fc2)

    # Fold LN affines into the consuming matmuls (exact algebra, fp64 on host).
    Wa_eff = (ln1_g.astype(np.float64)[:, None] * W_attn).astype(np.float32)
    ba_eff = (b_attn.astype(np.float64)
              + ln1_b.astype(np.float64) @ W_attn).astype(np.float32)
    scale = 1.0 / np.sqrt(np.float32(HD))
    WQn = (Wa_eff[:, 0:D] * scale).astype(np.float32)
    BQn = (ba_eff[0:D] * scale).astype(np.float32)
    WKn, BKn = Wa_eff[:, D:2 * D].copy(), ba_eff[D:2 * D].copy()
    WVn, BVn = Wa_eff[:, 2 * D:3 * D].copy(), ba_eff[2 * D:3 * D].copy()
    Wf_eff = (ln2_g.astype(np.float64)[:, None] * W_fc).astype(np.float32)
    bf_eff = (b_fc.astype(np.float64)
              + ln2_b.astype(np.float64) @ W_fc).astype(np.float32)

    shared = {
        "WQ": np.ascontiguousarray(WQn),
        "WK": np.ascontiguousarray(WKn),
        "WV": np.ascontiguousarray(WVn),
        "BQ": np.ascontiguousarray(BQn[:, None]),
        "BK": np.ascontiguousarray(BKn[:, None]),
        "BV": np.ascontiguousarray(BVn[None, :]),
        "WP": np.ascontiguousarray(W_proj),
        "BP": np.ascontiguousarray(b_proj[None, :]),
        "WF": np.ascontiguousarray(Wf_eff),
        "BF": np.ascontiguousarray(bf_eff[:, None]),
        "WF2": np.ascontiguousarray(W_fc2),
        "BF2": np.ascontiguousarray(b_fc2[None, :]),
    }

    in_maps, perms = [], []
    for c in range(N_CORES):
        b, f = c >> 1, c & 1
        perm = _perm_for(f)
        perms.append(perm)
        x_ctx = np.ascontiguousarray(hidden_states[b][perm])
        gk = perm
        gq = perm[OWN:]
        causal = np.where(gk[:, None] <= gq[None, :], np.float32(0.0),
                          np.float32(MASKED_BIAS))
        am = attention_mask[b, 0, 0, :].astype(np.float32)
        am_zero = bool(np.all(attention_mask == 0))
        if am_zero:
            # pair tiles: [:, u*QG:(u+1)*QG] is k-block (base+u) vs q-group g
            # boundary pair (other-parity k blocks 2g, 2g+1) is g-independent
            msk = np.zeros((2, 128, 512), np.float32)
            g = 0
            for u, j in enumerate([2 * g, 2 * g + 1]):
                msk[0, :, u * QG:(u + 1) * QG] = causal[
                    j * 128:(j + 1) * 128, g * QG:(g + 1) * QG]
            for u, j in enumerate([8 + 2 * g, 9 + 2 * g]):
                msk[1, :, u * QG:(u + 1) * QG] = causal[
                    j * 128:(j + 1) * 128, g * QG:(g + 1) * QG]
        else:
            msk = np.empty((16, 128, QG), np.float32)
            for g in range(NQG):
                for rel, j in enumerate([2 * g, 2 * g + 1,
                                         8 + 2 * g, 9 + 2 * g]):
                    msk[g * 4 + rel] = causal[j * 128:(j + 1) * 128,
                                              g * QG:(g + 1) * QG]
        am_t = np.ascontiguousarray(am[perm].reshape(SB, 128).T)
        in_maps.append({"X": x_ctx, "MSK": np.ascontiguousarray(msk),
                        "AM": am_t, **shared})
    return in_maps, perms


def kernel(hidden_states, attention_mask, ln1_g, ln1_b, W_attn, b_attn,
           W_proj, b_proj, ln2_g, ln2_b, W_fc, b_fc, W_fc2, b_fc2):
    in_maps, perms = make_in_maps(
        hidden_states, attention_mask, ln1_g, ln1_b, W_attn, b_attn,
        W_proj, b_proj, ln2_g, ln2_b, W_fc, b_fc, W_fc2, b_fc2)
    am_zero = bool(np.all(np.asarray(attention_mask) == 0))
    nc = _get_nc(am_zero)
    res = run_bass_kernel_spmd(nc, in_maps, core_ids=list(range(N_CORES)))
    out = np.empty((B, S, D), dtype=np.float32)
    for c in range(N_CORES):
        b = c >> 1
        out[b][perms[c][OWN:]] = res.results[c]["OUT"]
    return out


# revision 26
# speedup vs baseline: 1.0574x; 1.0574x over previous
"""Fused GPT-2 transformer block on 8 Trainium2 NeuronCores.

Sharding: 8 cores = 4 batches x 2 causal-balanced folds. Core (b, f) owns the 8
interleaved 128-token blocks of parity f of batch b (queries), and receives all
2048 tokens of batch b as context, permuted [other-parity blocks | own blocks].
Causality is enforced by a per-core additive mask shipped as data, so a single
SPMD program serves all cores. No collectives.

Layouts: LN1(x) is PE-transposed to hT [D, tok]; Q/K are produced directly in
head-major transposed layout, V in token-major layout with an appended ones
column (so the P@V matmul also accumulates softmax denominators). Attention
runs fully in the transposed layout; proj/fc2 contract against feature-major
lhsT slices, landing outputs back in token-major layout for residuals/LN.
All matmuls run in float32r (full PE rate, ~1.5e-4 rel err).
"""

import contextlib
import os

import numpy as np

import concourse.bass as bass
import concourse.mybir as mybir
import concourse.tile as tile
from concourse import bacc
from concourse.bass_utils import run_bass_kernel_spmd
from concourse.masks import make_identity

F32 = mybir.dt.float32
F32R = mybir.dt.float32r
AF = mybir.ActivationFunctionType
ALU = mybir.AluOpType

B, S, D, H = 4, 2048, 1024, 16
HD = D // H          # 64
DFF = 4 * D          # 4096
EPS = 1e-5
MASKED_BIAS = -10000.0
N_CORES = 8

SB = S // 128        # 16 ctx blocks
OWN = S // 2         # 1024 own tokens
OB = OWN // 128      # 8 own blocks
NQG = 4              # q-groups of 256
QG = 256
HSETS = 4            # head sets
HPS = H // HSETS     # 4 heads per set


def _klist(g):
    """ctx k-block indices computed for q-group g (own blocks 2g, 2g+1)."""
    return list(range(0, 2 * g + 2)) + list(range(8, 8 + 2 * g + 2))


def build_nc(am_zero=True):
    nc = bacc.Bacc("TRN2", target_bir_lowering=False, debug=False,
                   num_devices=N_CORES)

    X = nc.dram_tensor("X", [S, D], F32, kind="ExternalInput")
    MSK = (nc.dram_tensor("MSK", [2, 128, 512], F32, kind="ExternalInput")
           if am_zero else
           nc.dram_tensor("MSK", [16, 128, QG], F32, kind="ExternalInput"))
    AM = nc.dram_tensor("AM", [128, SB], F32, kind="ExternalInput")
    WQ = nc.dram_tensor("WQ", [D, D], F32, kind="ExternalInput")
    WK = nc.dram_tensor("WK", [D, D], F32, kind="ExternalInput")
    WV = nc.dram_tensor("WV", [D, D], F32, kind="ExternalInput")
    BQ = nc.dram_tensor("BQ", [D, 1], F32, kind="ExternalInput")
    BK = nc.dram_tensor("BK", [D, 1], F32, kind="ExternalInput")
    BV = nc.dram_tensor("BV", [1, D], F32, kind="ExternalInput")
    WP = nc.dram_tensor("WP", [D, D], F32, kind="ExternalInput")
    BP = nc.dram_tensor("BP", [1, D], F32, kind="ExternalInput")
    WF = nc.dram_tensor("WF", [D, DFF], F32, kind="ExternalInput")
    BF = nc.dram_tensor("BF", [DFF, 1], F32, kind="ExternalInput")
    WF2 = nc.dram_tensor("WF2", [DFF, D], F32, kind="ExternalInput")
    BF2 = nc.dram_tensor("BF2", [1, D], F32, kind="ExternalInput")
    OUT = nc.dram_tensor("OUT", [OWN, D], F32, kind="ExternalOutput")

    with tile.TileContext(nc) as tc:
        _body(nc, tc, X, MSK, AM, WQ, WK, WV, BQ, BK, BV, WP, BP, WF, BF,
              WF2, BF2, OUT, am_zero)
    nc.compile()
    return nc


def _layernorm_tile(nc, stat, src_tile):
    """In-place LN (no affine) of src_tile [128, D]."""
    sub = 512
    nsub = D // sub
    xs = src_tile[:, :].rearrange("p (n s) -> p n s", s=sub)
    stats = stat.tile([128, nsub, nc.vector.BN_STATS_DIM], F32, tag="bnst")
    for j in range(nsub):
        nc.vector.bn_stats(out=stats[:, j, :], in_=xs[:, j, :])
    mv = stat.tile([128, nc.vector.BN_AGGR_DIM], F32, tag="bnag")
    nc.vector.bn_aggr(out=mv[:, :], in_=stats[:, :, :])
    eps_t = stat.tile([128, 1], F32, tag="eps")
    nc.vector.memset(eps_t[:], EPS)
    nc.scalar.activation(out=mv[:, 1:2], in_=mv[:, 1:2], func=AF.Sqrt,
                         bias=eps_t[:], scale=1.0)
    nc.vector.reciprocal(out=mv[:, 1:2], in_=mv[:, 1:2])
    nc.vector.tensor_scalar(out=src_tile[:, :], in0=src_tile[:, :],
                            scalar1=mv[:, 0:1], scalar2=mv[:, 1:2],
                            op0=ALU.subtract, op1=ALU.mult)


def _body(nc, tc, X, MSK, AM, WQ, WK, WV, BQ, BK, BV, WP, BP, WF, BF, WF2,
          BF2, OUT, am_zero=True):
    PL = int(os.environ.get("KPHASES", "9"))
    CH = 8  # exp-staging chunk, in k-tiles
    with contextlib.ExitStack() as top:
        cst = top.enter_context(tc.tile_pool(name="cst", bufs=1))
        stat = top.enter_context(tc.tile_pool(name="stat", bufs=2))

        ident = cst.tile([128, 128], F32)
        make_identity(nc, ident[:])
        ones_f = cst.tile([1, 128], F32)
        nc.vector.memset(ones_f[:], 1.0)
        ones_c3 = cst.tile([128, HPS, 1], F32)
        nc.vector.memset(ones_c3[:], 1.0)
        ones_r = cst.tile([1, 128], F32R)   # bias-row lhsT
        nc.scalar.copy(ones_r[:], ones_f[:])
        ones_b = cst.tile([1, 64], F32R)    # denominator-broadcast lhsT
        nc.scalar.copy(ones_b[:], ones_f[:, 0:64])

        with contextlib.ExitStack() as attn_stack:
            atp = attn_stack.enter_context(tc.tile_pool(name="atp", bufs=1))
            aT = [atp.tile([128, OWN], F32R, tag=f"aT{p}", name=f"aT{p}")
                  for p in range(8)]

            with contextlib.ExitStack() as ht_stack:
                ht = ht_stack.enter_context(tc.tile_pool(name="ht", bufs=1))

                # ---- Phase 1: LN1 over ctx + transpose -> hT ----
                hT = [ht.tile([128, S], F32R, tag=f"hT{db}", name=f"hT{db}")
                      for db in range(8)]
                with tc.tile_pool(name="psT", bufs=4, space="PSUM") as psT, \
                     tc.tile_pool(name="xin1", bufs=3) as xin:
                    for tb in range(SB):
                        x_t = xin.tile([128, D], F32, tag="x1")
                        nc.sync.dma_start(x_t[:], X[tb * 128:(tb + 1) * 128, :])
                        _layernorm_tile(nc, stat, x_t)
                        for db in range(8):
                            pt = psT.tile([128, 128], F32, tag="tp")
                            nc.tensor.transpose(
                                pt[:], x_t[:, db * 128:(db + 1) * 128],
                                ident[:])
                            nc.vector.tensor_copy(
                                hT[db][:, tb * 128:(tb + 1) * 128], pt[:])

                if PL < 2:
                    return
                with contextlib.ExitStack() as hs_stack:
                    kvq = hs_stack.enter_context(
                        tc.tile_pool(name="kvq", bufs=1))
                    mskp = hs_stack.enter_context(
                        tc.tile_pool(name="mskp", bufs=1))
                    att = hs_stack.enter_context(
                        tc.tile_pool(name="att", bufs=2))
                    wst = hs_stack.enter_context(
                        tc.tile_pool(name="wstA", bufs=1))
                    psKV = hs_stack.enter_context(
                        tc.tile_pool(name="psKV", bufs=2, space="PSUM"))
                    psS = hs_stack.enter_context(
                        tc.tile_pool(name="psS", bufs=2, space="PSUM"))
                    psA = hs_stack.enter_context(
                        tc.tile_pool(name="psA", bufs=2, space="PSUM"))

                    # masks + attention-mask bias, loaded once
                    if am_zero:
                        mskB = mskp.tile([128, 512], F32, tag="mskB",
                                         name="mskB")
                        nc.sync.dma_start(mskB[:], MSK[0, :, :])
                        mskD = mskp.tile([128, 512], F32, tag="mskD",
                                         name="mskD")
                        nc.sync.dma_start(mskD[:], MSK[1, :, :])
                        msk_t = None
                    else:
                        msk_t = []
                        for m in range(16):
                            m_t = mskp.tile([128, QG], F32, tag=f"msk{m}",
                                            name=f"msk{m}")
                            nc.sync.dma_start(m_t[:], MSK[m, :, :])
                            msk_t.append(m_t)
                    am_sb = mskp.tile([128, SB], F32, tag="am", name="am")
                    nc.sync.dma_start(am_sb[:], AM[:, :])

                    for hs in range(HSETS):
                        # ---- Phase 2: K/V/Q projections for this head set ----
                        kT = [kvq.tile([128, S], F32R, tag=f"kT{p}",
                                       name=f"kT{p}") for p in range(2)]
                        qT = [kvq.tile([128, OWN], F32R, tag=f"qT{p}",
                                       name=f"qT{p}") for p in range(2)]
                        vS = [kvq.tile([128, HPS, HD + 1], F32R,
                                       tag=f"vS{tb}", name=f"vS{tb}")
                              for tb in range(SB)]

                        for p in range(2):
                            fcol = hs * 256 + p * 128
                            bq_c = stat.tile([128, 1], F32, tag="bqc")
                            nc.sync.dma_start(bq_c[:], BQ[fcol:fcol + 128, :])
                            bk_c = stat.tile([128, 1], F32, tag="bkc")
                            nc.sync.dma_start(bk_c[:], BK[fcol:fcol + 128, :])
                            wkt, wqt = [], []
                            for db in range(8):
                                w_t = wst.tile([128, 128], F32R,
                                               tag=f"wk{db}", name=f"wk{db}")
                                nc.sync.dma_start(
                                    w_t[:],
                                    WK[db * 128:(db + 1) * 128,
                                       fcol:fcol + 128].bitcast(F32R))
                                wkt.append(w_t)
                                w_t = wst.tile([128, 128], F32R,
                                               tag=f"wq{db}", name=f"wq{db}")
                                nc.sync.dma_start(
                                    w_t[:],
                                    WQ[db * 128:(db + 1) * 128,
                                       fcol:fcol + 128].bitcast(F32R))
                                wqt.append(w_t)
                            for tg in range(4):
                                ps = psKV.tile([128, 512], F32, tag="pk")
                                for db in range(8):
                                    nc.tensor.matmul(
                                        ps[:], wkt[db][:],
                                        hT[db][:, tg * 512:(tg + 1) * 512],
                                        start=(db == 0), stop=(db == 7))
                                nc.vector.tensor_scalar_add(
                                    out=kT[p][:, tg * 512:(tg + 1) * 512],
                                    in0=ps[:], scalar1=bk_c[:])
                            for tg in range(2):
                                ps = psKV.tile([128, 512], F32, tag="pk")
                                for db in range(8):
                                    nc.tensor.matmul(
                                        ps[:], wqt[db][:],
                                        hT[db][:, OWN + tg * 512:
                                               OWN + (tg + 1) * 512],
                                        start=(db == 0), stop=(db == 7))
                                nc.vector.tensor_scalar_add(
                                    out=qT[p][:, tg * 512:(tg + 1) * 512],
                                    in0=ps[:], scalar1=bq_c[:])

                        wvt = []
                        for db in range(8):
                            w_t = wst.tile([128, HPS * HD], F32R,
                                           tag=f"wv{db}", name=f"wv{db}")
                            nc.sync.dma_start(
                                w_t[:],
                                WV[db * 128:(db + 1) * 128,
                                   hs * 256:(hs + 1) * 256].bitcast(F32R))
                            wvt.append(w_t)
                        bv_t = wst.tile([1, HPS * HD], F32R, tag="bv")
                        nc.sync.dma_start(
                            bv_t[:],
                            BV[0:1, hs * 256:(hs + 1) * 256].bitcast(F32R))
                        for tb in range(SB):
                            ps = psKV.tile([128, HPS * HD], F32, tag="pv")
                            for db in range(8):
                                nc.tensor.matmul(
                                    ps[:], hT[db][:, tb * 128:(tb + 1) * 128],
                                    wvt[db][:], start=(db == 0), stop=False)
                            nc.tensor.matmul(ps[:], ones_r[:], bv_t[:],
                                             start=False, stop=True)
                            nc.vector.tensor_copy(
                                vS[tb][:, :, 0:HD],
                                ps[:].rearrange("p (h d) -> p h d", d=HD))
                            nc.scalar.copy(vS[tb][:, :, HD:HD + 1],
                                           ones_c3[:])

                        # ---- Phase 3: attention for this head set ----
                        for g in range(NQG):
                            kl = _klist(g)
                            # masked k-tiles for this q-group -> MSK index
                            mrel = {2 * g: 0, 2 * g + 1: 1,
                                    8 + 2 * g: 2, 9 + 2 * g: 3}
                            for h in range(HPS):
                                p, sub = h // 2, h % 2
                                pa = psA.tile([HD + 1, QG], F32, tag="pa")
                                nchunks = (len(kl) + CH - 1) // CH
                                for c in range(nchunks):
                                    chunk = kl[c * CH:(c + 1) * CH]
                                    wide = att.tile([128, CH * QG], F32R,
                                                    tag="wide", name="wide")
                                    if am_zero:
                                        for pi in range(len(chunk) // 2):
                                            kb0 = chunk[2 * pi]
                                            pss = psS.tile([128, 2 * QG], F32,
                                                           tag="ps")
                                            for u in range(2):
                                                kb = chunk[2 * pi + u]
                                                nc.tensor.matmul(
                                                    pss[:, u * QG:
                                                        (u + 1) * QG],
                                                    kT[p][sub * 64:
                                                          (sub + 1) * 64,
                                                          kb * 128:
                                                          (kb + 1) * 128],
                                                    qT[p][sub * 64:
                                                          (sub + 1) * 64,
                                                          g * QG:(g + 1) * QG],
                                                    start=True, stop=True)
                                            ws = wide[:, 2 * pi * QG:
                                                      (2 * pi + 2) * QG]
                                            if kb0 == 2 * g:
                                                nc.vector.scalar_tensor_tensor(
                                                    out=ws, in0=pss[:],
                                                    scalar=0.0, in1=mskB[:],
                                                    op0=ALU.add, op1=ALU.add)
                                            elif kb0 == 8 + 2 * g:
                                                nc.vector.scalar_tensor_tensor(
                                                    out=ws, in0=pss[:],
                                                    scalar=0.0, in1=mskD[:],
                                                    op0=ALU.add, op1=ALU.add)
                                            else:
                                                nc.vector.tensor_copy(
                                                    ws, pss[:])
                                    else:
                                        for i, kb in enumerate(chunk):
                                            pss = psS.tile([128, QG], F32,
                                                           tag="ps")
                                            nc.tensor.matmul(
                                                pss[:],
                                                kT[p][sub * 64:(sub + 1) * 64,
                                                      kb * 128:
                                                      (kb + 1) * 128],
                                                qT[p][sub * 64:(sub + 1) * 64,
                                                      g * QG:(g + 1) * QG],
                                                start=True, stop=True)
                                            wslice = wide[:, i * QG:
                                                          (i + 1) * QG]
                                            if kb in mrel:
                                                nc.vector.scalar_tensor_tensor(
                                                    out=wslice, in0=pss[:],
                                                    scalar=am_sb[:, kb:kb + 1],
                                                    in1=msk_t[g * 4
                                                              + mrel[kb]][:],
                                                    op0=ALU.add, op1=ALU.add)
                                            else:
                                                nc.vector.tensor_scalar_add(
                                                    out=wslice, in0=pss[:],
                                                    scalar1=am_sb[:,
                                                                  kb:kb + 1])
                                    nw = len(chunk) * QG
                                    nc.scalar.activation(
                                        wide[:, 0:nw],
                                        wide[:, 0:nw].bitcast(F32), AF.Exp)
                                    for i, kb in enumerate(chunk):
                                        nc.tensor.matmul(
                                            pa[:], vS[kb][:, h, :],
                                            wide[:, i * QG:(i + 1) * QG],
                                            start=(c == 0 and i == 0),
                                            stop=(c == nchunks - 1
                                                  and i == len(chunk) - 1))
                                rec = att.tile([1, QG], F32R, tag="rec")
                                with nc.allow_low_precision(
                                        reason="softmax denom reciprocal, "
                                               "f32r is ~fp32"):
                                    nc.vector.reciprocal(rec[:],
                                                         pa[HD:HD + 1, :])
                                pb = psS.tile([64, QG], F32, tag="ps")
                                nc.tensor.matmul(pb[:], ones_b[:], rec[:],
                                                 start=True, stop=True)
                                bc = att.tile([64, QG], F32, tag="bc")
                                nc.scalar.copy(bc[:], pb[:])
                                ap_idx = 2 * hs + p
                                nc.vector.tensor_mul(
                                    aT[ap_idx][sub * 64:(sub + 1) * 64,
                                               g * QG:(g + 1) * QG],
                                    pa[0:HD, :], bc[:])

            if PL < 4:
                return
            # ---- Phase 4: proj + residual -> x2 ----
            x2p = top.enter_context(tc.tile_pool(name="x2p", bufs=1,
                                                 side="right"))
            x2 = [x2p.tile([128, D], F32, tag=f"x2{tb}", name=f"x2{tb}")
                  for tb in range(OB)]
            with tc.tile_pool(name="psP", bufs=2, space="PSUM") as psP, \
                 tc.tile_pool(name="wstP", bufs=1) as wst, \
                 tc.tile_pool(name="xqp", bufs=1) as xqp:
                xqs = []
                for tb in range(OB):
                    xq_t = xqp.tile([128, D], F32, tag=f"xq{tb}",
                                    name=f"xq{tb}")
                    nc.sync.dma_start(
                        xq_t[:], X[OWN + tb * 128:OWN + (tb + 1) * 128, :])
                    xqs.append(xq_t)
                for fg in range(2):
                    wpt = []
                    for ab in range(8):
                        w_t = wst.tile([128, 512], F32R, tag=f"wp{ab}",
                                       name=f"wp{ab}")
                        nc.sync.dma_start(
                            w_t[:],
                            WP[ab * 128:(ab + 1) * 128,
                               fg * 512:(fg + 1) * 512].bitcast(F32R))
                        wpt.append(w_t)
                    bp_t = wst.tile([1, 512], F32R, tag="bp")
                    nc.sync.dma_start(
                        bp_t[:],
                        BP[0:1, fg * 512:(fg + 1) * 512].bitcast(F32R))
                    for tb in range(OB):
                        ps = psP.tile([128, 512], F32, tag="pp")
                        for ab in range(8):
                            nc.tensor.matmul(
                                ps[:], aT[ab][:, tb * 128:(tb + 1) * 128],
                                wpt[ab][:], start=(ab == 0), stop=False)
                        nc.tensor.matmul(ps[:], ones_r[:], bp_t[:],
                                         start=False, stop=True)
                        nc.vector.tensor_add(
                            x2[tb][:, fg * 512:(fg + 1) * 512], ps[:],
                            xqs[tb][:, fg * 512:(fg + 1) * 512])

        if PL < 5:
            return
        # ---- Phase 5: LN2 + transpose -> h2T ----
        with contextlib.ExitStack() as mlp_stack:
            ht2 = mlp_stack.enter_context(tc.tile_pool(name="ht2", bufs=1))
            h2T = [ht2.tile([128, OWN], F32R, tag=f"h2T{db}", name=f"h2T{db}")
                   for db in range(8)]
            with tc.tile_pool(name="psT2", bufs=4, space="PSUM") as psT2, \
                 tc.tile_pool(name="xin5", bufs=3) as xin:
                for tb in range(OB):
                    x_t = xin.tile([128, D], F32, tag="x1")
                    nc.vector.tensor_copy(x_t[:], x2[tb][:])
                    _layernorm_tile(nc, stat, x_t)
                    for db in range(8):
                        pt = psT2.tile([128, 128], F32, tag="tp")
                        nc.tensor.transpose(
                            pt[:], x_t[:, db * 128:(db + 1) * 128], ident[:])
                        nc.vector.tensor_copy(
                            h2T[db][:, tb * 128:(tb + 1) * 128], pt[:])

            if PL < 6:
                return
            # ---- Phase 6: MLP ----
            with contextlib.ExitStack() as mlp2:
                gtp = mlp2.enter_context(tc.tile_pool(name="gtp", bufs=1))
                wst = mlp2.enter_context(tc.tile_pool(name="wstF", bufs=2))
                wst6 = mlp2.enter_context(tc.tile_pool(name="wstF6", bufs=6))
                outp = mlp2.enter_context(tc.tile_pool(name="outp", bufs=3))
                psF = mlp2.enter_context(
                    tc.tile_pool(name="psF", bufs=2, space="PSUM"))
                psO = mlp2.enter_context(
                    tc.tile_pool(name="psO", bufs=1, space="PSUM"))
                for tg in range(2):
                    gt = [gtp.tile([128, 512], F32R, tag=f"gt{j}",
                                   name=f"gt{j}") for j in range(32)]
                    for jj in range(8):      # groups of 4 dff blocks
                        slabs = []
                        for db in range(8):
                            w_t = wst.tile([128, 512], F32R, tag=f"wf{db}",
                                           name=f"wf{db}")
                            nc.sync.dma_start(
                                w_t[:],
                                WF[db * 128:(db + 1) * 128,
                                   jj * 512:(jj + 1) * 512].bitcast(F32R))
                            slabs.append(w_t)
                        for sj in range(4):
                            j = jj * 4 + sj
                            bf_c = stat.tile([128, 1], F32, tag="bfc")
                            nc.sync.dma_start(bf_c[:],
                                              BF[j * 128:(j + 1) * 128, :])
                            ps = psF.tile([128, 512], F32, tag="pf")
                            for db in range(8):
                                nc.tensor.matmul(
                                    ps[:],
                                    slabs[db][:, sj * 128:(sj + 1) * 128],
                                    h2T[db][:, tg * 512:(tg + 1) * 512],
                                    start=(db == 0), stop=(db == 7))
                            nc.scalar.activation(gt[j][:], ps[:],
                                                 AF.Gelu_apprx_tanh,
                                                 bias=bf_c[:], scale=1.0)
                    for fg in range(2):
                        pso = [psO.tile([128, 512], F32, tag=f"po{tb}",
                                        name=f"po{tb}") for tb in range(4)]
                        for j in range(32):
                            w_t = wst6.tile([128, 512], F32R, tag="wf2",
                                            name="wf2")
                            nc.sync.dma_start(
                                w_t[:],
                                WF2[j * 128:(j + 1) * 128,
                                    fg * 512:(fg + 1) * 512].bitcast(F32R))
                            for tb in range(4):
                                nc.tensor.matmul(
                                    pso[tb][:],
                                    gt[j][:, tb * 128:(tb + 1) * 128],
                                    w_t[:], start=(j == 0), stop=False)
                        bf2_t = wst.tile([1, 512], F32R, tag="bf2")
                        nc.sync.dma_start(
                            bf2_t[:],
                            BF2[0:1, fg * 512:(fg + 1) * 512].bitcast(F32R))
                        for tb in range(4):
                            nc.tensor.matmul(pso[tb][:], ones_r[:], bf2_t[:],
                                             start=False, stop=True)
                            o_t = outp.tile([128, 512], F32, tag="ot")
                            gtb = tg * 4 + tb
                            nc.vector.tensor_add(
                                o_t[:], pso[tb][:],
                                x2[gtb][:, fg * 512:(fg + 1) * 512])
                            nc.sync.dma_start(
                                OUT[gtb * 128:(gtb + 1) * 128,
                                    fg * 512:(fg + 1) * 512], o_t[:])


_NC_CACHE = {}


def _get_nc(am_zero=True):
    key = f"nc{int(am_zero)}"
    if key not in _NC_CACHE:
        _NC_CACHE[key] = build_nc(am_zero)
    return _NC_CACHE[key]


def _perm_for(f):
    other = [2 * j + (1 - f) for j in range(8)]
    own = [2 * j + f for j in range(8)]
    blocks = other + own
    return np.concatenate([np.arange(b * 128, (b + 1) * 128) for b in blocks])


def make_in_maps(hidden_states, attention_mask, ln1_g, ln1_b, W_attn, b_attn,
                 W_proj, b_proj, ln2_g, ln2_b, W_fc, b_fc, W_fc2, b_fc2):
    f32 = lambda a: np.asarray(a, dtype=np.float32)
    hidden_states = f32(hidden_states)
    attention_mask = f32(attention_mask)
    ln1_g, ln1_b = f32(ln1_g), f32(ln1_b)
    ln2_g, ln2_b = f32(ln2_g), f32(ln2_b)
    W_attn, b_attn = f32(W_attn), f32(b_attn)
    W_proj, b_proj = f32(W_proj), f32(b_proj)
    W_fc, b_fc = f32(W_fc), f32(b_fc)
    W_fc2, b_fc2 = f32(W_fc2), f32(b_fc2)

    # Fold LN affines into the consuming matmuls (exact algebra, fp64 on host).
    Wa_eff = (ln1_g.astype(np.float64)[:, None] * W_attn).astype(np.float32)
    ba_eff = (b_attn.astype(np.float64)
              + ln1_b.astype(np.float64) @ W_attn).astype(np.float32)
    scale = 1.0 / np.sqrt(np.float32(HD))
    WQn = (Wa_eff[:, 0:D] * scale).astype(np.float32)
    BQn = (ba_eff[0:D] * scale).astype(np.float32)
    WKn, BKn = Wa_eff[:, D:2 * D].copy(), ba_eff[D:2 * D].copy()
    WVn, BVn = Wa_eff[:, 2 * D:3 * D].copy(), ba_eff[2 * D:3 * D].copy()
    Wf_eff = (ln2_g.astype(np.float64)[:, None] * W_fc).astype(np.float32)
    bf_eff = (b_fc.astype(np.float64)
              + ln2_b.astype(np.float64) @ W_fc).astype(np.float32)

    shared = {
        "WQ": np.ascontiguousarray(WQn),
        "WK": np.ascontiguousarray(WKn),
        "WV": np.ascontiguousarray(WVn),
        "BQ": np.ascontiguousarray(BQn[:, None]),
        "BK": np.ascontiguousarray(BKn[:, None]),
        "BV": np.ascontiguousarray(BVn[None, :]),
        "WP": np.ascontiguousarray(W_proj),
        "BP": np.ascontiguousarray(b_proj[None, :]),
        "WF": np.ascontiguousarray(Wf_eff),
        "BF": np.ascontiguousarray(bf_eff[:, None]),
        "WF2": np.ascontiguousarray(W_fc2),
        "BF2": np.ascontiguousarray(b_fc2[None, :]),
    }

    in_maps, perms = [], []
    for c in range(N_CORES):
        b, f = c >> 1, c & 1
        perm = _perm_for(f)
        perms.append(perm)
        x_ctx = np.ascontiguousarray(hidden_states[b][perm])
        gk = perm
        gq = perm[OWN:]
        causal = np.where(gk[:, None] <= gq[None, :], np.float32(0.0),
                          np.float32(MASKED_BIAS))
        am = attention_mask[b, 0, 0, :].astype(np.float32)
        am_zero = bool(np.all(attention_mask == 0))
        if am_zero:
            # pair tiles: [:, u*QG:(u+1)*QG] is k-block (base+u) vs q-group g
            # boundary pair (other-parity k blocks 2g, 2g+1) is g-independent
            msk = np.zeros((2, 128, 512), np.float32)
            g = 0
            for u, j in enumerate([2 * g, 2 * g + 1]):
                msk[0, :, u * QG:(u + 1) * QG] = causal[
                    j * 128:(j + 1) * 128, g * QG:(g + 1) * QG]
            for u, j in enumerate([8 + 2 * g, 9 + 2 * g]):
                msk[1, :, u * QG:(u + 1) * QG] = causal[
                    j * 128:(j + 1) * 128, g * QG:(g + 1) * QG]
        else:
            msk = np.empty((16, 128, QG), np.float32)
            for g in range(NQG):
                for rel, j in enumerate([2 * g, 2 * g + 1,
                                         8 + 2 * g, 9 + 2 * g]):
                    msk[g * 4 + rel] = causal[j * 128:(j + 1) * 128,
                                              g * QG:(g + 1) * QG]
        am_t = np.ascontiguousarray(am[perm].reshape(SB, 128).T)
        in_maps.append({"X": x_ctx, "MSK": np.ascontiguousarray(msk),
                        "AM": am_t, **shared})
    return in_maps, perms


def kernel(hidden_states, attention_mask, ln1_g, ln1_b, W_attn, b_attn,
           W_proj, b_proj, ln2_g, ln2_b, W_fc, b_fc, W_fc2, b_fc2):
    in_maps, perms = make_in_maps(
        hidden_states, attention_mask, ln1_g, ln1_b, W_attn, b_attn,
        W_proj, b_proj, ln2_g, ln2_b, W_fc, b_fc, W_fc2, b_fc2)
    am_zero = bool(np.all(np.asarray(attention_mask) == 0))
    nc = _get_nc(am_zero)
    res = run_bass_kernel_spmd(nc, in_maps, core_ids=list(range(N_CORES)))
    out = np.empty((B, S, D), dtype=np.float32)
    for c in range(N_CORES):
        b = c >> 1
        out[b][perms[c][OWN:]] = res.results[c]["OUT"]
    return out


# revision 29
# speedup vs baseline: 1.1811x; 1.1171x over previous
"""Fused GPT-2 transformer block on 8 Trainium2 NeuronCores.

Sharding: 8 cores = 4 batches x 2 causal-balanced folds. Core (b, f) owns the 8
interleaved 128-token blocks of parity f of batch b (queries), and receives all
2048 tokens of batch b as context, permuted [other-parity blocks | own blocks].
Causality is enforced by a per-core additive mask shipped as data, so a single
SPMD program serves all cores. No collectives.

Layouts: LN1(x) is PE-transposed to hT [D, tok]; Q/K are produced directly in
head-major transposed layout, V in token-major layout with an appended ones
column (so the P@V matmul also accumulates softmax denominators). Attention
runs fully in the transposed layout; proj/fc2 contract against feature-major
lhsT slices, landing outputs back in token-major layout for residuals/LN.
All matmuls run in float32r (full PE rate, ~1.5e-4 rel err).
"""

import contextlib
import os

import numpy as np

import concourse.bass as bass
import concourse.mybir as mybir
import concourse.tile as tile
from concourse import bacc
from concourse.bass_utils import run_bass_kernel_spmd
from concourse.masks import make_identity

F32 = mybir.dt.float32
F32R = mybir.dt.float32r
AF = mybir.ActivationFunctionType
ALU = mybir.AluOpType

B, S, D, H = 4, 2048, 1024, 16
HD = D // H          # 64
DFF = 4 * D          # 4096
EPS = 1e-5
MASKED_BIAS = -10000.0
N_CORES = 8

SB = S // 128        # 16 ctx blocks
OWN = S // 2         # 1024 own tokens
OB = OWN // 128      # 8 own blocks
NQG = 4              # q-groups of 256
QG = 256
HSETS = 4            # head sets
HPS = H // HSETS     # 4 heads per set


def _klist(g):
    """ctx k-block indices computed for q-group g (own blocks 2g, 2g+1)."""
    return list(range(0, 2 * g + 2)) + list(range(8, 8 + 2 * g + 2))


def build_nc(am_zero=True):
    nc = bacc.Bacc("TRN2", target_bir_lowering=False, debug=False,
                   num_devices=N_CORES)

    X = nc.dram_tensor("X", [S, D], F32, kind="ExternalInput")
    MSK = (nc.dram_tensor("MSK", [2, 128, 512], F32, kind="ExternalInput")
           if am_zero else
           nc.dram_tensor("MSK", [16, 128, QG], F32, kind="ExternalInput"))
    AM = nc.dram_tensor("AM", [128, SB], F32, kind="ExternalInput")
    WQ = nc.dram_tensor("WQ", [D, D], F32, kind="ExternalInput")
    WK = nc.dram_tensor("WK", [D, D], F32, kind="ExternalInput")
    WV = nc.dram_tensor("WV", [D, D], F32, kind="ExternalInput")
    BQ = nc.dram_tensor("BQ", [D, 1], F32, kind="ExternalInput")
    BK = nc.dram_tensor("BK", [D, 1], F32, kind="ExternalInput")
    BV = nc.dram_tensor("BV", [1, D], F32, kind="ExternalInput")
    WP = nc.dram_tensor("WP", [D, D], F32, kind="ExternalInput")
    BP = nc.dram_tensor("BP", [1, D], F32, kind="ExternalInput")
    WF = nc.dram_tensor("WF", [D, DFF], F32, kind="ExternalInput")
    BF = nc.dram_tensor("BF", [DFF, 1], F32, kind="ExternalInput")
    WF2 = nc.dram_tensor("WF2", [DFF, D], F32, kind="ExternalInput")
    BF2 = nc.dram_tensor("BF2", [1, D], F32, kind="ExternalInput")
    OUT = nc.dram_tensor("OUT", [OWN, D], F32, kind="ExternalOutput")

    with tile.TileContext(nc) as tc:
        _body(nc, tc, X, MSK, AM, WQ, WK, WV, BQ, BK, BV, WP, BP, WF, BF,
              WF2, BF2, OUT, am_zero)
    nc.compile()
    return nc


def _layernorm_tile(nc, stat, src_tile, eps_t):
    """In-place LN (no affine) of src_tile [128, D]."""
    sub = 512
    nsub = D // sub
    xs = src_tile[:, :].rearrange("p (n s) -> p n s", s=sub)
    stats = stat.tile([128, nsub, nc.vector.BN_STATS_DIM], F32, tag="bnst")
    for j in range(nsub):
        nc.vector.bn_stats(out=stats[:, j, :], in_=xs[:, j, :])
    mv = stat.tile([128, nc.vector.BN_AGGR_DIM], F32, tag="bnag")
    nc.vector.bn_aggr(out=mv[:, :], in_=stats[:, :, :])
    nc.scalar.activation(out=mv[:, 1:2], in_=mv[:, 1:2], func=AF.Sqrt,
                         bias=eps_t[:], scale=1.0)
    nc.vector.reciprocal(out=mv[:, 1:2], in_=mv[:, 1:2])
    nc.vector.tensor_scalar(out=src_tile[:, :], in0=src_tile[:, :],
                            scalar1=mv[:, 0:1], scalar2=mv[:, 1:2],
                            op0=ALU.subtract, op1=ALU.mult)


def _body(nc, tc, X, MSK, AM, WQ, WK, WV, BQ, BK, BV, WP, BP, WF, BF, WF2,
          BF2, OUT, am_zero=True):
    PL = int(os.environ.get("KPHASES", "9"))
    CH = 8  # exp-staging chunk, in k-tiles
    with contextlib.ExitStack() as top:
        cst = top.enter_context(tc.tile_pool(name="cst", bufs=1))
        stat = top.enter_context(tc.tile_pool(name="stat", bufs=4))

        ident = cst.tile([128, 128], F32)
        make_identity(nc, ident[:])
        ones_f = cst.tile([1, 128], F32)
        nc.vector.memset(ones_f[:], 1.0)
        ones_c3 = cst.tile([128, HPS, 1], F32)
        nc.vector.memset(ones_c3[:], 1.0)
        ones_r = cst.tile([1, 128], F32R)   # bias-row lhsT
        nc.scalar.copy(ones_r[:], ones_f[:])
        ones_b = cst.tile([1, 64], F32R)    # denominator-broadcast lhsT
        nc.scalar.copy(ones_b[:], ones_f[:, 0:64])
        eps_t = cst.tile([128, 1], F32)
        nc.vector.memset(eps_t[:], EPS)

        with contextlib.ExitStack() as attn_stack:
            atp = attn_stack.enter_context(tc.tile_pool(name="atp", bufs=1))
            aT = [atp.tile([128, OWN], F32R, tag=f"aT{p}", name=f"aT{p}")
                  for p in range(8)]

            with contextlib.ExitStack() as ht_stack:
                ht = ht_stack.enter_context(tc.tile_pool(name="ht", bufs=1))

                # ---- Phase 1: LN1 over ctx + transpose -> hT ----
                hT = [ht.tile([128, S], F32R, tag=f"hT{db}", name=f"hT{db}")
                      for db in range(8)]
                with tc.tile_pool(name="psT", bufs=4, space="PSUM") as psT, \
                     tc.tile_pool(name="xin1", bufs=3) as xin:
                    for tb in range(SB):
                        x_t = xin.tile([128, D], F32, tag="x1")
                        nc.sync.dma_start(x_t[:], X[tb * 128:(tb + 1) * 128, :])
                        _layernorm_tile(nc, stat, x_t, eps_t)
                        for db in range(8):
                            pt = psT.tile([128, 128], F32, tag="tp")
                            nc.tensor.transpose(
                                pt[:], x_t[:, db * 128:(db + 1) * 128],
                                ident[:])
                            nc.vector.tensor_copy(
                                hT[db][:, tb * 128:(tb + 1) * 128], pt[:])

                if PL < 2:
                    return
                with contextlib.ExitStack() as hs_stack:
                    kvq = hs_stack.enter_context(
                        tc.tile_pool(name="kvq", bufs=1))
                    mskp = hs_stack.enter_context(
                        tc.tile_pool(name="mskp", bufs=1))
                    att = hs_stack.enter_context(
                        tc.tile_pool(name="att", bufs=3))
                    wst = hs_stack.enter_context(
                        tc.tile_pool(name="wstA", bufs=1))
                    psKV = hs_stack.enter_context(
                        tc.tile_pool(name="psKV", bufs=2, space="PSUM"))
                    psS = hs_stack.enter_context(
                        tc.tile_pool(name="psS", bufs=2, space="PSUM"))
                    psA = hs_stack.enter_context(
                        tc.tile_pool(name="psA", bufs=2, space="PSUM"))

                    # masks + attention-mask bias, loaded once
                    if am_zero:
                        mskB = mskp.tile([128, 512], F32, tag="mskB",
                                         name="mskB")
                        nc.sync.dma_start(mskB[:], MSK[0, :, :])
                        mskD = mskp.tile([128, 512], F32, tag="mskD",
                                         name="mskD")
                        nc.sync.dma_start(mskD[:], MSK[1, :, :])
                        msk_t = None
                    else:
                        msk_t = []
                        for m in range(16):
                            m_t = mskp.tile([128, QG], F32, tag=f"msk{m}",
                                            name=f"msk{m}")
                            nc.sync.dma_start(m_t[:], MSK[m, :, :])
                            msk_t.append(m_t)
                    am_sb = mskp.tile([128, SB], F32, tag="am", name="am")
                    nc.sync.dma_start(am_sb[:], AM[:, :])

                    for hs in range(HSETS):
                        # ---- Phase 2: K/V/Q projections for this head set ----
                        kT = [kvq.tile([128, S], F32R, tag=f"kT{p}",
                                       name=f"kT{p}") for p in range(2)]
                        qT = [kvq.tile([128, OWN], F32R, tag=f"qT{p}",
                                       name=f"qT{p}") for p in range(2)]
                        vS = [kvq.tile([128, HPS, HD + 1], F32R,
                                       tag=f"vS{tb}", name=f"vS{tb}")
                              for tb in range(SB)]

                        for p in range(2):
                            fcol = hs * 256 + p * 128
                            bq_c = stat.tile([128, 1], F32, tag="bqc")
                            nc.sync.dma_start(bq_c[:], BQ[fcol:fcol + 128, :])
                            bk_c = stat.tile([128, 1], F32, tag="bkc")
                            nc.sync.dma_start(bk_c[:], BK[fcol:fcol + 128, :])
                            wkt, wqt = [], []
                            for db in range(8):
                                w_t = wst.tile([128, 128], F32R,
                                               tag=f"wk{db}", name=f"wk{db}")
                                nc.sync.dma_start(
                                    w_t[:],
                                    WK[db * 128:(db + 1) * 128,
                                       fcol:fcol + 128].bitcast(F32R))
                                wkt.append(w_t)
                                w_t = wst.tile([128, 128], F32R,
                                               tag=f"wq{db}", name=f"wq{db}")
                                nc.sync.dma_start(
                                    w_t[:],
                                    WQ[db * 128:(db + 1) * 128,
                                       fcol:fcol + 128].bitcast(F32R))
                                wqt.append(w_t)
                            for tg in range(4):
                                ps = psKV.tile([128, 512], F32, tag="pk")
                                for db in range(8):
                                    nc.tensor.matmul(
                                        ps[:], wkt[db][:],
                                        hT[db][:, tg * 512:(tg + 1) * 512],
                                        start=(db == 0), stop=(db == 7))
                                nc.vector.tensor_scalar_add(
                                    out=kT[p][:, tg * 512:(tg + 1) * 512],
                                    in0=ps[:], scalar1=bk_c[:])
                            for tg in range(2):
                                ps = psKV.tile([128, 512], F32, tag="pk")
                                for db in range(8):
                                    nc.tensor.matmul(
                                        ps[:], wqt[db][:],
                                        hT[db][:, OWN + tg * 512:
                                               OWN + (tg + 1) * 512],
                                        start=(db == 0), stop=(db == 7))
                                nc.vector.tensor_scalar_add(
                                    out=qT[p][:, tg * 512:(tg + 1) * 512],
                                    in0=ps[:], scalar1=bq_c[:])

                        wvt = []
                        for db in range(8):
                            w_t = wst.tile([128, HPS * HD], F32R,
                                           tag=f"wv{db}", name=f"wv{db}")
                            nc.sync.dma_start(
                                w_t[:],
                                WV[db * 128:(db + 1) * 128,
                                   hs * 256:(hs + 1) * 256].bitcast(F32R))
                            wvt.append(w_t)
                        bv_t = wst.tile([1, HPS * HD], F32R, tag="bv")
                        nc.sync.dma_start(
                            bv_t[:],
                            BV[0:1, hs * 256:(hs + 1) * 256].bitcast(F32R))
                        for tb in range(SB):
                            ps = psKV.tile([128, HPS * HD], F32, tag="pv")
                            for db in range(8):
                                nc.tensor.matmul(
                                    ps[:], hT[db][:, tb * 128:(tb + 1) * 128],
                                    wvt[db][:], start=(db == 0), stop=False)
                            nc.tensor.matmul(ps[:], ones_r[:], bv_t[:],
                                             start=False, stop=True)
                            nc.vector.tensor_copy(
                                vS[tb][:, :, 0:HD],
                                ps[:].rearrange("p (h d) -> p h d", d=HD))
                            nc.scalar.copy(vS[tb][:, :, HD:HD + 1],
                                           ones_c3[:])

                        # ---- Phase 3: attention for this head set ----
                        for g in range(NQG):
                            kl = _klist(g)
                            # masked k-tiles for this q-group -> MSK index
                            mrel = {2 * g: 0, 2 * g + 1: 1,
                                    8 + 2 * g: 2, 9 + 2 * g: 3}
                            for h in range(HPS):
                                p, sub = h // 2, h % 2
                                pa = psA.tile([HD + 1, QG], F32, tag="pa")
                                nchunks = (len(kl) + CH - 1) // CH
                                for c in range(nchunks):
                                    chunk = kl[c * CH:(c + 1) * CH]
                                    wide = att.tile([128, CH * QG], F32R,
                                                    tag="wide", name="wide")
                                    if am_zero:
                                        for pi in range(len(chunk) // 2):
                                            kb0 = chunk[2 * pi]
                                            pss = psS.tile([128, 2 * QG], F32,
                                                           tag="ps")
                                            for u in range(2):
                                                kb = chunk[2 * pi + u]
                                                nc.tensor.matmul(
                                                    pss[:, u * QG:
                                                        (u + 1) * QG],
                                                    kT[p][sub * 64:
                                                          (sub + 1) * 64,
                                                          kb * 128:
                                                          (kb + 1) * 128],
                                                    qT[p][sub * 64:
                                                          (sub + 1) * 64,
                                                          g * QG:(g + 1) * QG],
                                                    start=True, stop=True)
                                            ws = wide[:, 2 * pi * QG:
                                                      (2 * pi + 2) * QG]
                                            if kb0 == 2 * g:
                                                nc.vector.scalar_tensor_tensor(
                                                    out=ws, in0=pss[:],
                                                    scalar=0.0, in1=mskB[:],
                                                    op0=ALU.add, op1=ALU.add)
                                                nc.scalar.activation(
                                                    ws, ws.bitcast(F32),
                                                    AF.Exp)
                                            elif kb0 == 8 + 2 * g:
                                                nc.vector.scalar_tensor_tensor(
                                                    out=ws, in0=pss[:],
                                                    scalar=0.0, in1=mskD[:],
                                                    op0=ALU.add, op1=ALU.add)
                                                nc.scalar.activation(
                                                    ws, ws.bitcast(F32),
                                                    AF.Exp)
                                            else:
                                                nc.scalar.activation(
                                                    ws, pss[:], AF.Exp)
                                    else:
                                        for i, kb in enumerate(chunk):
                                            pss = psS.tile([128, QG], F32,
                                                           tag="ps")
                                            nc.tensor.matmul(
                                                pss[:],
                                                kT[p][sub * 64:(sub + 1) * 64,
                                                      kb * 128:
                                                      (kb + 1) * 128],
                                                qT[p][sub * 64:(sub + 1) * 64,
                                                      g * QG:(g + 1) * QG],
                                                start=True, stop=True)
                                            wslice = wide[:, i * QG:
                                                          (i + 1) * QG]
                                            if kb in mrel:
                                                nc.vector.scalar_tensor_tensor(
                                                    out=wslice, in0=pss[:],
                                                    scalar=am_sb[:, kb:kb + 1],
                                                    in1=msk_t[g * 4
                                                              + mrel[kb]][:],
                                                    op0=ALU.add, op1=ALU.add)
                                            else:
                                                nc.vector.tensor_scalar_add(
                                                    out=wslice, in0=pss[:],
                                                    scalar1=am_sb[:,
                                                                  kb:kb + 1])
                                    if not am_zero:
                                        nw = len(chunk) * QG
                                        nc.scalar.activation(
                                            wide[:, 0:nw],
                                            wide[:, 0:nw].bitcast(F32),
                                            AF.Exp)
                                    for i, kb in enumerate(chunk):
                                        nc.tensor.matmul(
                                            pa[:], vS[kb][:, h, :],
                                            wide[:, i * QG:(i + 1) * QG],
                                            start=(c == 0 and i == 0),
                                            stop=(c == nchunks - 1
                                                  and i == len(chunk) - 1))
                                rec = att.tile([1, QG], F32R, tag="rec")
                                with nc.allow_low_precision(
                                        reason="softmax denom reciprocal, "
                                               "f32r is ~fp32"):
                                    nc.vector.reciprocal(rec[:],
                                                         pa[HD:HD + 1, :])
                                pb = psS.tile([64, QG], F32, tag="ps")
                                nc.tensor.matmul(pb[:], ones_b[:], rec[:],
                                                 start=True, stop=True)
                                bc = att.tile([64, QG], F32, tag="bc")
                                nc.scalar.copy(bc[:], pb[:])
                                ap_idx = 2 * hs + p
                                nc.vector.tensor_mul(
                                    aT[ap_idx][sub * 64:(sub + 1) * 64,
                                               g * QG:(g + 1) * QG],
                                    pa[0:HD, :], bc[:])

            if PL < 4:
                return
            # ---- Phase 4: proj + residual -> x2 ----
            x2p = top.enter_context(tc.tile_pool(name="x2p", bufs=1,
                                                 side="right"))
            x2 = [x2p.tile([128, D], F32, tag=f"x2{tb}", name=f"x2{tb}")
                  for tb in range(OB)]
            with tc.tile_pool(name="psP", bufs=2, space="PSUM") as psP, \
                 tc.tile_pool(name="wstP", bufs=1) as wst, \
                 tc.tile_pool(name="xqp", bufs=1) as xqp:
                xqs = []
                for tb in range(OB):
                    xq_t = xqp.tile([128, D], F32, tag=f"xq{tb}",
                                    name=f"xq{tb}")
                    nc.sync.dma_start(
                        xq_t[:], X[OWN + tb * 128:OWN + (tb + 1) * 128, :])
                    xqs.append(xq_t)
                for fg in range(2):
                    wpt = []
                    for ab in range(8):
                        w_t = wst.tile([128, 512], F32R, tag=f"wp{ab}",
                                       name=f"wp{ab}")
                        nc.sync.dma_start(
                            w_t[:],
                            WP[ab * 128:(ab + 1) * 128,
                               fg * 512:(fg + 1) * 512].bitcast(F32R))
                        wpt.append(w_t)
                    bp_t = wst.tile([1, 512], F32R, tag="bp")
                    nc.sync.dma_start(
                        bp_t[:],
                        BP[0:1, fg * 512:(fg + 1) * 512].bitcast(F32R))
                    for tb in range(OB):
                        ps = psP.tile([128, 512], F32, tag="pp")
                        for ab in range(8):
                            nc.tensor.matmul(
                                ps[:], aT[ab][:, tb * 128:(tb + 1) * 128],
                                wpt[ab][:], start=(ab == 0), stop=False)
                        nc.tensor.matmul(ps[:], ones_r[:], bp_t[:],
                                         start=False, stop=True)
                        nc.vector.tensor_add(
                            x2[tb][:, fg * 512:(fg + 1) * 512], ps[:],
                            xqs[tb][:, fg * 512:(fg + 1) * 512])

        if PL < 5:
            return
        # ---- Phase 5: LN2 + transpose -> h2T ----
        with contextlib.ExitStack() as mlp_stack:
            ht2 = mlp_stack.enter_context(tc.tile_pool(name="ht2", bufs=1))
            h2T = [ht2.tile([128, OWN], F32R, tag=f"h2T{db}", name=f"h2T{db}")
                   for db in range(8)]
            with tc.tile_pool(name="psT2", bufs=4, space="PSUM") as psT2, \
                 tc.tile_pool(name="xin5", bufs=3) as xin:
                for tb in range(OB):
                    x_t = xin.tile([128, D], F32, tag="x1")
                    nc.vector.tensor_copy(x_t[:], x2[tb][:])
                    _layernorm_tile(nc, stat, x_t, eps_t)
                    for db in range(8):
                        pt = psT2.tile([128, 128], F32, tag="tp")
                        nc.tensor.transpose(
                            pt[:], x_t[:, db * 128:(db + 1) * 128], ident[:])
                        nc.vector.tensor_copy(
                            h2T[db][:, tb * 128:(tb + 1) * 128], pt[:])

            if PL < 6:
                return
            # ---- Phase 6: MLP ----
            with contextlib.ExitStack() as mlp2:
                gtp = mlp2.enter_context(tc.tile_pool(name="gtp", bufs=1))
                wst = mlp2.enter_context(tc.tile_pool(name="wstF", bufs=2))
                wst6 = mlp2.enter_context(tc.tile_pool(name="wstF6", bufs=6))
                outp = mlp2.enter_context(tc.tile_pool(name="outp", bufs=3))
                psF = mlp2.enter_context(
                    tc.tile_pool(name="psF", bufs=2, space="PSUM"))
                psO = mlp2.enter_context(
                    tc.tile_pool(name="psO", bufs=1, space="PSUM"))
                for tg in range(2):
                    gt = [gtp.tile([128, 512], F32R, tag=f"gt{j}",
                                   name=f"gt{j}") for j in range(32)]
                    for jj in range(8):      # groups of 4 dff blocks
                        slabs = []
                        for db in range(8):
                            w_t = wst.tile([128, 512], F32R, tag=f"wf{db}",
                                           name=f"wf{db}")
                            nc.sync.dma_start(
                                w_t[:],
                                WF[db * 128:(db + 1) * 128,
                                   jj * 512:(jj + 1) * 512].bitcast(F32R))
                            slabs.append(w_t)
                        for sj in range(4):
                            j = jj * 4 + sj
                            bf_c = stat.tile([128, 1], F32, tag="bfc")
                            nc.sync.dma_start(bf_c[:],
                                              BF[j * 128:(j + 1) * 128, :])
                            ps = psF.tile([128, 512], F32, tag="pf")
                            for db in range(8):
                                nc.tensor.matmul(
                                    ps[:],
                                    slabs[db][:, sj * 128:(sj + 1) * 128],
                                    h2T[db][:, tg * 512:(tg + 1) * 512],
                                    start=(db == 0), stop=(db == 7))
                            nc.scalar.activation(gt[j][:], ps[:],
                                                 AF.Gelu_apprx_tanh,
                                                 bias=bf_c[:], scale=1.0)
                    for fg in range(2):
                        pso = [psO.tile([128, 512], F32, tag=f"po{tb}",
                                        name=f"po{tb}") for tb in range(4)]
                        for j in range(32):
                            w_t = wst6.tile([128, 512], F32R, tag="wf2",
                                            name="wf2")
                            nc.sync.dma_start(
                                w_t[:],
                                WF2[j * 128:(j + 1) * 128,
                                    fg * 512:(fg + 1) * 512].bitcast(F32R))
                            for tb in range(4):
                                nc.tensor.matmul(
                                    pso[tb][:],
                                    gt[j][:, tb * 128:(tb + 1) * 128],
                                    w_t[:], start=(j == 0), stop=False)
                        bf2_t = wst.tile([1, 512], F32R, tag="bf2")
                        nc.sync.dma_start(
                            bf2_t[:],
                            BF2[0:1, fg * 512:(fg + 1) * 512].bitcast(F32R))
                        for tb in range(4):
                            nc.tensor.matmul(pso[tb][:], ones_r[:], bf2_t[:],
                                             start=False, stop=True)
                            o_t = outp.tile([128, 512], F32, tag="ot")
                            gtb = tg * 4 + tb
                            nc.vector.tensor_add(
                                o_t[:], pso[tb][:],
                                x2[gtb][:, fg * 512:(fg + 1) * 512])
                            nc.sync.dma_start(
                                OUT[gtb * 128:(gtb + 1) * 128,
                                    fg * 512:(fg + 1) * 512], o_t[:])


_NC_CACHE = {}


def _get_nc(am_zero=True):
    key = f"nc{int(am_zero)}"
    if key not in _NC_CACHE:
        _NC_CACHE[key] = build_nc(am_zero)
    return _NC_CACHE[key]


def _perm_for(f):
    other = [2 * j + (1 - f) for j in range(8)]
    own = [2 * j + f for j in range(8)]
    blocks = other + own
    return np.concatenate([np.arange(b * 128, (b + 1) * 128) for b in blocks])


def make_in_maps(hidden_states, attention_mask, ln1_g, ln1_b, W_attn, b_attn,
                 W_proj, b_proj, ln2_g, ln2_b, W_fc, b_fc, W_fc2, b_fc2):
    f32 = lambda a: np.asarray(a, dtype=np.float32)
    hidden_states = f32(hidden_states)
    attention_mask = f32(attention_mask)
    ln1_g, ln1_b = f32(ln1_g), f32(ln1_b)
    ln2_g, ln2_b = f32(ln2_g), f32(ln2_b)
    W_attn, b_attn = f32(W_attn), f32(b_attn)
    W_proj, b_proj = f32(W_proj), f32(b_proj)
    W_fc, b_fc = f32(W_fc), f32(b_fc)
    W_fc2, b_fc2 = f32(W_fc2), f32(b_fc2)

    # Fold LN affines into the consuming matmuls (exact algebra, fp64 on host).
    Wa_eff = (ln1_g.astype(np.float64)[:, None] * W_attn).astype(np.float32)
    ba_eff = (b_attn.astype(np.float64)
              + ln1_b.astype(np.float64) @ W_attn).astype(np.float32)
    scale = 1.0 / np.sqrt(np.float32(HD))
    WQn = (Wa_eff[:, 0:D] * scale).astype(np.float32)
    BQn = (ba_eff[0:D] * scale).astype(np.float32)
    WKn, BKn = Wa_eff[:, D:2 * D].copy(), ba_eff[D:2 * D].copy()
    WVn, BVn = Wa_eff[:, 2 * D:3 * D].copy(), ba_eff[2 * D:3 * D].copy()
    Wf_eff = (ln2_g.astype(np.float64)[:, None] * W_fc).astype(np.float32)
    bf_eff = (b_fc.astype(np.float64)
              + ln2_b.astype(np.float64) @ W_fc).astype(np.float32)

    shared = {
        "WQ": np.ascontiguousarray(WQn),
        "WK": np.ascontiguousarray(WKn),
        "WV": np.ascontiguousarray(WVn),
        "BQ": np.ascontiguousarray(BQn[:, None]),
        "BK": np.ascontiguousarray(BKn[:, None]),
        "BV": np.ascontiguousarray(BVn[None, :]),
        "WP": np.ascontiguousarray(W_proj),
        "BP": np.ascontiguousarray(b_proj[None, :]),
        "WF": np.ascontiguousarray(Wf_eff),
        "BF": np.ascontiguousarray(bf_eff[:, None]),
        "WF2": np.ascontiguousarray(W_fc2),
        "BF2": np.ascontiguousarray(b_fc2[None, :]),
    }

    in_maps, perms = [], []
    for c in range(N_CORES):
        b, f = c >> 1, c & 1
        perm = _perm_for(f)
        perms.append(perm)
        x_ctx = np.ascontiguousarray(hidden_states[b][perm])
        gk = perm
        gq = perm[OWN:]
        causal = np.where(gk[:, None] <= gq[None, :], np.float32(0.0),
                          np.float32(MASKED_BIAS))
        am = attention_mask[b, 0, 0, :].astype(np.float32)
        am_zero = bool(np.all(attention_mask == 0))
        if am_zero:
            # pair tiles: [:, u*QG:(u+1)*QG] is k-block (base+u) vs q-group g
            # boundary pair (other-parity k blocks 2g, 2g+1) is g-independent
            msk = np.zeros((2, 128, 512), np.float32)
            g = 0
            for u, j in enumerate([2 * g, 2 * g + 1]):
                msk[0, :, u * QG:(u + 1) * QG] = causal[
                    j * 128:(j + 1) * 128, g * QG:(g + 1) * QG]
            for u, j in enumerate([8 + 2 * g, 9 + 2 * g]):
                msk[1, :, u * QG:(u + 1) * QG] = causal[
                    j * 128:(j + 1) * 128, g * QG:(g + 1) * QG]
        else:
            msk = np.empty((16, 128, QG), np.float32)
            for g in range(NQG):
                for rel, j in enumerate([2 * g, 2 * g + 1,
                                         8 + 2 * g, 9 + 2 * g]):
                    msk[g * 4 + rel] = causal[j * 128:(j + 1) * 128,
                                              g * QG:(g + 1) * QG]
        am_t = np.ascontiguousarray(am[perm].reshape(SB, 128).T)
        in_maps.append({"X": x_ctx, "MSK": np.ascontiguousarray(msk),
                        "AM": am_t, **shared})
    return in_maps, perms


def kernel(hidden_states, attention_mask, ln1_g, ln1_b, W_attn, b_attn,
           W_proj, b_proj, ln2_g, ln2_b, W_fc, b_fc, W_fc2, b_fc2):
    in_maps, perms = make_in_maps(
        hidden_states, attention_mask, ln1_g, ln1_b, W_attn, b_attn,
        W_proj, b_proj, ln2_g, ln2_b, W_fc, b_fc, W_fc2, b_fc2)
    am_zero = bool(np.all(np.asarray(attention_mask) == 0))
    nc = _get_nc(am_zero)
    res = run_bass_kernel_spmd(nc, in_maps, core_ids=list(range(N_CORES)))
    out = np.empty((B, S, D), dtype=np.float32)
    for c in range(N_CORES):
        b = c >> 1
        out[b][perms[c][OWN:]] = res.results[c]["OUT"]
    return out
